# revision 7
# baseline (speedup 1.0000x reference)
# Trainium2 Bass kernel for nn_DecoderMHA (dense decoder multi-head attention).
#
# Sharding (8 NeuronCores): batch (4) x tensor-parallel over heads (2).
# Core c handles batch b = c//2 and heads [tp*8, tp*8+8) where tp = c%2.
# Per-core partial outputs are summed on the host (y[b] = part0 + part1 + bo).
#
# Per-core pipeline (matmul operands bf16, fp32 PSUM accumulation):
#   A) Q^T/K^T [512,2048] and V [2048,512] projections from x^T.
#   B) Attention with HEAD-PAIRED scores: heads 2t and 2t+1 live on
#      partitions 0-63 / 64-127 of qT[t]/kT[t], so their K=64 score matmuls
#      run CONCURRENTLY on the PE array via row tiling (auto tile_position
#      from base partitions 0/64).  Both heads' score^T tiles share one
#      [128,1024] PSUM strip (head even -> bank cols 0:512, head odd ->
#      512:1024); ONE ScalarE exp covers both heads via a strided AP.
#      Causal mask multiply on diagonal blocks only (DVE, bf16).
#      attn@V uses a v-augmented-with-ones lhsT so PSUM row 64 is the
#      softmax denominator.  Normalisation: DVE reciprocal straight from
#      PSUM row 64, SBUF->SBUF broadcast DMA across partitions, one
#      multiply per head.  No DRAM round trips.
#   C) y = out @ Wo^T partial, emitted per 512-query block one round late
#      so it overlaps the next block's attention.  y stored as bf16.
import os
import numpy as np

BSZ, SEQ, DM = 4, 2048, 1024
HEADS, DK = 16, 64
NCORES, TP = 8, 2
E = DM // TP          # 512 per-core projection slice
HPC = HEADS // TP     # 8 heads per core
NPAIR = HPC // 2      # 4 head pairs per core
P = 128
NDC = DM // P         # 8 contraction chunks
NSC = SEQ // P        # 16 sequence chunks
NSB = SEQ // 512      # 4 sequence blocks (x tiling)
NQB = SEQ // 512      # 4 query blocks
SCALE = 1.0 / float(np.sqrt(DK))

_CACHED = {}


def _split_sync_waits(nc, mybir, max_waits=1):
    """The walrus in this container only accepts one sync-wait per
    instruction; move excess waits onto NoOps in front."""
    n = 0
    for fn in nc.m.functions:
        for bb in fn.blocks:
            insts = bb.instructions
            i = 0
            while i < len(insts):
                inst = insts[i]
                si = getattr(inst, "sync_info", None)
                if si is not None and si.on_wait and len(si.on_wait) > max_waits:
                    waits = list(si.on_wait)
                    extra, keep = waits[:-max_waits], waits[-max_waits:]
                    si.on_wait = keep
                    pos = i
                    for j in range(0, len(extra), max_waits):
                        nop = mybir.InstNoOp(
                            name=nc.get_next_instruction_name(),
                            sync_info=mybir.SyncInfo(
                                on_wait=extra[j:j + max_waits], on_update=[]),
                            bass_nofuse=True,
                            engine=inst.engine,
                        )
                        insts.insert(pos, nop)
                        pos += 1
                        i += 1
                        n += 1
                i += 1
    return n


def _build():
    import concourse.bass as bass
    from concourse import mybir
    from concourse.tile import TileContext

    f32 = mybir.dt.float32
    bf16 = mybir.dt.bfloat16
    Exp = mybir.ActivationFunctionType.Exp
    MUL = mybir.AluOpType.mult
    ADD = mybir.AluOpType.add

    nc = bass.Bass("TRN2", target_bir_lowering=False, debug=False,
                   num_devices=NCORES)

    # DRAM I/O (per-core layouts, pre-tiled on host)
    xt = nc.dram_tensor("xt", [NSB, P, NDC, 512], bf16, kind="ExternalInput")
    wq = nc.dram_tensor("wq", [P, NDC, E], bf16, kind="ExternalInput")
    wk = nc.dram_tensor("wk", [P, NDC, E], bf16, kind="ExternalInput")
    wv = nc.dram_tensor("wv", [P, NDC, E], bf16, kind="ExternalInput")
    wo = nc.dram_tensor("wo", [P, 4, DM], bf16, kind="ExternalInput")
    bqt = nc.dram_tensor("bqt", [P, 4], f32, kind="ExternalInput")
    bkt = nc.dram_tensor("bkt", [P, 4], f32, kind="ExternalInput")
    bvb = nc.dram_tensor("bvb", [P, E], f32, kind="ExternalInput")
    cm = nc.dram_tensor("cm", [P, P], bf16, kind="ExternalInput")
    pb = nc.dram_tensor("pb", [P, NSC], f32, kind="ExternalInput")
    ones = nc.dram_tensor("ones", [P, HPC], bf16, kind="ExternalInput")
    y = nc.dram_tensor("y", [SEQ, DM], bf16, kind="ExternalOutput")

    with TileContext(nc) as tc:
        with (
            tc.tile_pool(name="persist", bufs=1) as pp,
            tc.tile_pool(name="psS", bufs=2, space="PSUM") as psS,
            tc.tile_pool(name="psO", bufs=2, space="PSUM") as psO,
            tc.tile_pool(name="exp", bufs=4) as pe,
            tc.tile_pool(name="nrm", bufs=3) as pn,
            tc.tile_pool(name="scr", bufs=3, space="DRAM") as scr,
            tc.tile_pool(name="ys", bufs=2) as py,
        ):
            # ---- persistent SBUF ----
            qT = [pp.tile([P, SEQ], bf16, tag=f"qT{t}", name=f"qT{t}")
                  for t in range(NPAIR)]
            kT = [pp.tile([P, SEQ], bf16, tag=f"kT{t}", name=f"kT{t}")
                  for t in range(NPAIR)]
            vA = [pp.tile([P, HPC, DK + 1], bf16, tag=f"vA{g}", name=f"vA{g}")
                  for g in range(NSC)]
            outT = [pp.tile([P, SEQ], bf16, tag=f"oT{t}", name=f"oT{t}")
                    for t in range(NPAIR)]
            cm_s = pp.tile([P, P], bf16, tag="cm")
            pb_s = pp.tile([P, NSC], f32, tag="pb")
            bq_s = pp.tile([P, 4], f32, tag="bq")
            bk_s = pp.tile([P, 4], f32, tag="bk")
            bv_s = pp.tile([P, E], f32, tag="bv")

            # ---- input DMAs, ordered so compute can start earliest ----
            cm_dma = nc.sync.dma_start(cm_s[:], cm[:])
            pb_dma = nc.sync.dma_start(pb_s[:], pb[:])
            for g in range(NSC):
                nc.sync.dma_start(vA[g][:, :, DK:DK + 1], ones[:, :])
            wv_s = pp.tile([P, NDC, E], bf16, tag="wv")
            nc.sync.dma_start(wv_s[:], wv[:])
            xt_s = [pp.tile([P, NDC, 512], bf16, tag=f"xt{sb}",
                            name=f"xt{sb}") for sb in range(NSB)]
            nc.sync.dma_start(xt_s[0][:], xt[0])
            nc.sync.dma_start(bv_s[:], bvb[:])
            wq_s = pp.tile([P, NDC, E], bf16, tag="wq")
            wk_s = pp.tile([P, NDC, E], bf16, tag="wk")
            nc.sync.dma_start(wq_s[:], wq[:])
            nc.sync.dma_start(wk_s[:], wk[:])
            nc.sync.dma_start(bq_s[:], bqt[:])
            nc.sync.dma_start(bk_s[:], bkt[:])
            for sb in range(1, NSB):
                nc.sync.dma_start(xt_s[sb][:], xt[sb])
            wo_s = pp.tile([P, 4, DM], bf16, tag="wo")
            nc.sync.dma_start(wo_s[:], wo[:])

            def v_strip(g):
                """V projection for sequence chunk g -> vA[g] (+bias)."""
                sb, ssc = g // 4, g % 4
                psum = psS.tile([P, 512], f32, tag="strip", name=f"pv{g}")
                for dc in range(NDC):
                    nc.tensor.matmul(
                        psum[:],
                        xt_s[sb][:, dc, ssc * P:(ssc + 1) * P],
                        wv_s[:, dc, :],
                        start=(dc == 0), stop=(dc == NDC - 1))
                nc.vector.tensor_tensor(
                    vA[g][:, :, 0:DK],
                    psum[:].rearrange("p (h d) -> p h d", h=HPC),
                    bv_s[:].rearrange("p (h d) -> p h d", h=HPC),
                    ADD)

            def qk_proj(t):
                """Q^T and K^T rows for head pair t (partitions 0..127)."""
                for (w_s, b_s, dst, nm) in ((wq_s, bq_s, qT, "q"),
                                            (wk_s, bk_s, kT, "k")):
                    for sb in range(NSB):
                        psum = psS.tile([P, 512], f32, tag="strip",
                                        name=f"p{nm}{t}_{sb}")
                        for dc in range(NDC):
                            nc.tensor.matmul(
                                psum[:],
                                w_s[:, dc, t * P:(t + 1) * P],
                                xt_s[sb][:, dc, :],
                                start=(dc == 0), stop=(dc == NDC - 1))
                        nc.vector.tensor_tensor(
                            dst[t][:, sb * 512:(sb + 1) * 512],
                            psum[:],
                            b_s[:, t:t + 1].to_broadcast([P, 512]),
                            ADD)

            def attn_block(t, qb):
                """Attention for head pair t, query block qb (512 queries).

                Heads he=2t (partitions 0:64) and ho=2t+1 (64:128) run
                concurrently in the PE array (K=64 row tiling)."""
                q0 = qb * 512
                nkc = 4 * qb + 4
                ops_e = psO.tile([DK + 1, 512], f32, tag="opse",
                                 name=f"ope{t}_{qb}")
                ops_o = psO.tile([DK + 1, 512], f32, tag="opso",
                                 name=f"opo{t}_{qb}")
                for kc in range(nkc):
                    k0 = kc * P
                    c = max(0, k0 - q0)
                    w = 512 - c
                    strip = psS.tile([P, 1024], f32, tag="strip",
                                     name=f"st{t}_{qb}_{kc}")
                    # concurrent score matmuls (row groups 0-1 / 2-3)
                    nc.tensor.matmul(
                        strip[:, c:512],
                        kT[t][0:DK, k0:k0 + P],
                        qT[t][0:DK, q0 + c:q0 + 512],
                        start=True, stop=True)
                    nc.tensor.matmul(
                        strip[:, 512 + c:1024],
                        kT[t][DK:P, k0:k0 + P],
                        qT[t][DK:P, q0 + c:q0 + 512],
                        start=True, stop=True)
                    exp_s = pe.tile([P, 1024], bf16, tag="exp",
                                    name=f"ex{t}_{qb}_{kc}")
                    # one exp for both heads via strided [P, 2, w] APs
                    nc.scalar.activation(
                        exp_s[:].rearrange("p (h q) -> p h q", h=2)[:, :, c:512],
                        strip[:].rearrange("p (h q) -> p h q", h=2)[:, :, c:512],
                        Exp, bias=pb_s[:, kc:kc + 1], scale=SCALE)
                    if k0 >= q0:  # diagonal block: causal mask multiply
                        nc.vector.tensor_tensor(
                            exp_s[:, c:c + P], exp_s[:, c:c + P], cm_s[:], MUL)
                        nc.vector.tensor_tensor(
                            exp_s[:, 512 + c:512 + c + P],
                            exp_s[:, 512 + c:512 + c + P], cm_s[:], MUL)
                    nc.tensor.matmul(
                        ops_e[:, c:512], vA[kc][:, 2 * t, :], exp_s[:, c:512],
                        start=(kc == 0), stop=(kc == nkc - 1))
                    nc.tensor.matmul(
                        ops_o[:, c:512], vA[kc][:, 2 * t + 1, :],
                        exp_s[:, 512 + c:1024],
                        start=(kc == 0), stop=(kc == nkc - 1))
                # normalisation: reciprocal straight from PSUM row 64,
                # broadcast across partitions with an SBUF->SBUF DMA.
                rcp = pn.tile([P, 512], f32, tag="rcp", name=f"rc{t}_{qb}")
                nc.vector.reciprocal(rcp[0:1, :], ops_e[DK:DK + 1, :])
                nc.vector.reciprocal(rcp[DK:DK + 1, :], ops_o[DK:DK + 1, :])
                sc1 = scr.tile([2, 512], f32, tag="scr", name=f"sc{t}_{qb}")
                nc.gpsimd.dma_start(sc1[:], rcp[0:P:DK, :])
                bc = pn.tile([P, 512], f32, tag="bc", name=f"bc{t}_{qb}")
                nc.gpsimd.dma_start(bc[0:DK, :],
                                    sc1[0:1, :].to_broadcast([DK, 512]))
                nc.gpsimd.dma_start(bc[DK:P, :],
                                    sc1[1:2, :].to_broadcast([DK, 512]))
                nc.vector.tensor_tensor(
                    outT[t][0:DK, q0:q0 + 512], ops_e[0:DK, :], bc[0:DK, :],
                    MUL)
                nc.vector.tensor_tensor(
                    outT[t][DK:P, q0:q0 + 512], ops_o[0:DK, :], bc[DK:P, :],
                    MUL)

            def c_block(qb):
                """Output projection for query block qb (4 seq chunks)."""
                for j in range(4):
                    sc = qb * 4 + j
                    y_s = py.tile([P, DM], bf16, tag="ys", name=f"ys{sc}")
                    for eh in range(2):
                        psum = psS.tile([P, 512], f32, tag="strip",
                                        name=f"py{sc}_{eh}")
                        for t in range(NPAIR):
                            nc.tensor.matmul(
                                psum[:],
                                outT[t][:, sc * P:(sc + 1) * P],
                                wo_s[:, t, eh * 512:(eh + 1) * 512],
                                start=(t == 0), stop=(t == NPAIR - 1))
                        nc.vector.tensor_copy(
                            y_s[:, eh * 512:(eh + 1) * 512], psum[:])
                    nc.sync.dma_start(y[sc * P:(sc + 1) * P, :], y_s[:])

            # ---- emission order: maximise overlap ----
            for g in range(4):          # V for kc 0..3 (needs xt0 only)
                v_strip(g)
            for t in range(NPAIR):      # interleave QK proj with qb=0 attn
                qk_proj(t)
                attn_block(t, 0)
            for qb in range(1, NQB):
                for g in range(4 * qb, 4 * qb + 4):
                    v_strip(g)
                attn_block(0, qb)
                attn_block(1, qb)
                c_block(qb - 1)         # previous block's out-proj, now safe
                attn_block(2, qb)
                attn_block(3, qb)
            c_block(0)                  # qb=0 was skipped above
            c_block(NQB - 1)

    _split_sync_waits(nc, mybir)
    return nc


def _prep_inputs(x, pad_mask, Wq, bq, Wk, bk, Wv, bv, Wo, bo):
    """Build the 8 per-core input maps."""
    import ml_dtypes
    bf16 = ml_dtypes.bfloat16

    def tile3(a, n):  # [n*128, F] -> [128, n, F] in bf16
        return np.ascontiguousarray(
            a.reshape(n, P, a.shape[1]).transpose(1, 0, 2).astype(bf16))

    cmv = (np.arange(P)[:, None] <= np.arange(P)[None, :]).astype(bf16)
    in_maps = []
    for c in range(NCORES):
        b, tp = c // 2, c % 2
        sl = slice(tp * E, (tp + 1) * E)
        xT = np.ascontiguousarray(x[b].T.astype(np.float32))
        padb = np.where(pad_mask[b, 0, 0] == 1, -1e30, 0.0).astype(np.float32)
        in_maps.append({
            "xt": np.ascontiguousarray(
                tile3(xT, NDC).reshape(P, NDC, NSB, 512)
                .transpose(2, 0, 1, 3)),
            "wq": tile3(np.ascontiguousarray(Wq.T[:, sl]), NDC),
            "wk": tile3(np.ascontiguousarray(Wk.T[:, sl]), NDC),
            "wv": tile3(np.ascontiguousarray(Wv.T[:, sl]), NDC),
            "wo": tile3(np.ascontiguousarray(Wo.T[sl, :]), 4),
            "bqt": np.ascontiguousarray(bq[sl].reshape(4, P).T),
            "bkt": np.ascontiguousarray(bk[sl].reshape(4, P).T),
            "bvb": np.ascontiguousarray(np.tile(bv[sl][None, :], (P, 1))),
            "cm": cmv,
            "ones": np.ones((P, HPC), dtype=bf16),
            "pb": np.ascontiguousarray(padb.reshape(NSC, P).T),
        })
    return in_maps


def _enable_tracing():
    """Register the NTFF profile hook (the image lacks antenv.axon_hooks)
    and neuter the bucket upload the trace path attempts."""
    import sys
    import types
    try:
        import antenv.axon_hooks  # noqa: F401
    except ImportError:
        from trn_agent_boot.trn_boot import _ntff_profile_via_ctypes
        m = types.ModuleType("antenv.axon_hooks")
        hook = _ntff_profile_via_ctypes("/opt/axon/libaxon_pjrt.so")
        m.get_axon_ntff_profile_hook = lambda: hook
        sys.modules["antenv.axon_hooks"] = m
    import concourse.bass_utils as bu
    bu.upload_artifacts = lambda tmpdir: tmpdir


def kernel_with_stats(inputs, trace=False):
    from concourse.bass_utils import run_bass_kernel_spmd

    if trace:
        try:
            _enable_tracing()
        except Exception:
            trace = False

    if "nc" not in _CACHED:
        _CACHED["nc"] = _build()
    nc = _CACHED["nc"]
    in_maps = _prep_inputs(**inputs)
    res = run_bass_kernel_spmd(nc, in_maps, core_ids=list(range(NCORES)),
                               trace=trace)
    bo = inputs["bo"].astype(np.float32)
    out = np.empty((BSZ, SEQ, DM), dtype=np.float32)
    for b in range(BSZ):
        out[b] = (res.results[2 * b]["y"].astype(np.float32)
                  + res.results[2 * b + 1]["y"].astype(np.float32) + bo)
    return out, res


def kernel(**inputs):
    out, _ = kernel_with_stats(
        inputs, trace=bool(int(os.environ.get("KERNEL_TRACE", "0"))))
    return out


# revision 9
# speedup vs baseline: 1.2674x; 1.2674x over previous
# Trainium2 Bass kernel for nn_DecoderMHA (dense decoder multi-head attention).
#
# Sharding (8 NeuronCores): batch (4) x tensor-parallel over heads (2).
# Core c handles batch b = c//2 and heads [tp*8, tp*8+8) where tp = c%2.
# Per-core partial outputs are summed on the host (y[b] = part0 + part1 + bo).
#
# Per-core pipeline (matmul operands bf16, fp32 PSUM accumulation):
#   A) Q^T/K^T [512,2048] and V [2048,512] projections from x^T.
#   B) Attention with HEAD-PAIRED scores: heads 2t and 2t+1 live on
#      partitions 0-63 / 64-127 of qT[t]/kT[t], so their K=64 score matmuls
#      run CONCURRENTLY on the PE array via row tiling (auto tile_position
#      from base partitions 0/64).  Both heads' score^T tiles share one
#      [128,1024] PSUM strip (head even -> bank cols 0:512, head odd ->
#      512:1024); ONE ScalarE exp covers both heads via a strided AP.
#      Causal mask multiply on diagonal blocks only (DVE, bf16).
#      attn@V uses a v-augmented-with-ones lhsT so PSUM row 64 is the
#      softmax denominator.  Normalisation: DVE reciprocal straight from
#      PSUM row 64, SBUF->SBUF broadcast DMA across partitions, one
#      multiply per head.  No DRAM round trips.
#   C) y = out @ Wo^T partial, emitted per 512-query block one round late
#      so it overlaps the next block's attention.  y stored as bf16.
import os
import numpy as np

BSZ, SEQ, DM = 4, 2048, 1024
HEADS, DK = 16, 64
NCORES, TP = 8, 2
E = DM // TP          # 512 per-core projection slice
HPC = HEADS // TP     # 8 heads per core
NPAIR = HPC // 2      # 4 head pairs per core
P = 128
NDC = DM // P         # 8 contraction chunks
NSC = SEQ // P        # 16 sequence chunks
NSB = SEQ // 512      # 4 sequence blocks (x tiling)
NQB = SEQ // 512      # 4 query blocks
SCALE = 1.0 / float(np.sqrt(DK))

_CACHED = {}


def _split_sync_waits(nc, mybir, max_waits=1):
    """The walrus in this container only accepts one sync-wait per
    instruction; move excess waits onto NoOps in front."""
    n = 0
    for fn in nc.m.functions:
        for bb in fn.blocks:
            insts = bb.instructions
            i = 0
            while i < len(insts):
                inst = insts[i]
                si = getattr(inst, "sync_info", None)
                if si is not None and si.on_wait and len(si.on_wait) > max_waits:
                    waits = list(si.on_wait)
                    extra, keep = waits[:-max_waits], waits[-max_waits:]
                    si.on_wait = keep
                    pos = i
                    for j in range(0, len(extra), max_waits):
                        nop = mybir.InstNoOp(
                            name=nc.get_next_instruction_name(),
                            sync_info=mybir.SyncInfo(
                                on_wait=extra[j:j + max_waits], on_update=[]),
                            bass_nofuse=True,
                            engine=inst.engine,
                        )
                        insts.insert(pos, nop)
                        pos += 1
                        i += 1
                        n += 1
                i += 1
    return n


def _build():
    import concourse.bass as bass
    from concourse import mybir
    from concourse.tile import TileContext

    f32 = mybir.dt.float32
    bf16 = mybir.dt.bfloat16
    Exp = mybir.ActivationFunctionType.Exp
    MUL = mybir.AluOpType.mult
    ADD = mybir.AluOpType.add

    nc = bass.Bass("TRN2", target_bir_lowering=False, debug=False,
                   num_devices=NCORES)

    # DRAM I/O (per-core layouts, pre-tiled on host)
    xt = nc.dram_tensor("xt", [NSB, P, NDC, 512], bf16, kind="ExternalInput")
    wq = nc.dram_tensor("wq", [P, NDC, E], bf16, kind="ExternalInput")
    wk = nc.dram_tensor("wk", [P, NDC, E], bf16, kind="ExternalInput")
    wv = nc.dram_tensor("wv", [P, NDC, E], bf16, kind="ExternalInput")
    wo = nc.dram_tensor("wo", [P, 4, DM], bf16, kind="ExternalInput")
    bqt = nc.dram_tensor("bqt", [P, 4], f32, kind="ExternalInput")
    bkt = nc.dram_tensor("bkt", [P, 4], f32, kind="ExternalInput")
    bvb = nc.dram_tensor("bvb", [P, E], f32, kind="ExternalInput")
    cm = nc.dram_tensor("cm", [P, P], bf16, kind="ExternalInput")
    pb = nc.dram_tensor("pb", [P, NSC], f32, kind="ExternalInput")
    ones = nc.dram_tensor("ones", [P, HPC], bf16, kind="ExternalInput")
    y = nc.dram_tensor("y", [SEQ, DM], bf16, kind="ExternalOutput")

    with TileContext(nc) as tc:
        with (
            tc.tile_pool(name="persist", bufs=1) as pp,
            tc.tile_pool(name="psS", bufs=2, space="PSUM") as psS,
            tc.tile_pool(name="psO", bufs=2, space="PSUM") as psO,
            tc.tile_pool(name="exp", bufs=4) as pe,
            tc.tile_pool(name="nrm", bufs=3) as pn,
            tc.tile_pool(name="scr", bufs=3, space="DRAM") as scr,
            tc.tile_pool(name="ys", bufs=2) as py,
        ):
            # ---- persistent SBUF ----
            qT = [pp.tile([P, SEQ], bf16, tag=f"qT{t}", name=f"qT{t}")
                  for t in range(NPAIR)]
            kT = [pp.tile([P, SEQ], bf16, tag=f"kT{t}", name=f"kT{t}")
                  for t in range(NPAIR)]
            vA = [pp.tile([P, HPC, DK + 1], bf16, tag=f"vA{g}", name=f"vA{g}")
                  for g in range(NSC)]
            outT = [pp.tile([P, SEQ], bf16, tag=f"oT{t}", name=f"oT{t}")
                    for t in range(NPAIR)]
            cm_s = pp.tile([P, P], bf16, tag="cm")
            pb_s = pp.tile([P, NSC], f32, tag="pb")
            bq_s = pp.tile([P, 4], f32, tag="bq")
            bk_s = pp.tile([P, 4], f32, tag="bk")
            bv_s = pp.tile([P, E], f32, tag="bv")

            # ---- input DMAs, ordered so compute can start earliest ----
            wv_s = pp.tile([P, NDC, E], bf16, tag="wv")
            nc.sync.dma_start(wv_s[:], wv[:])
            xt_s = [pp.tile([P, NDC, 512], bf16, tag=f"xt{sb}",
                            name=f"xt{sb}") for sb in range(NSB)]
            nc.sync.dma_start(xt_s[0][:], xt[0])
            nc.sync.dma_start(bv_s[:], bvb[:])
            wq_s = pp.tile([P, NDC, E], bf16, tag="wq")
            wk_s = pp.tile([P, NDC, E], bf16, tag="wk")
            nc.sync.dma_start(wq_s[:], wq[:])
            nc.sync.dma_start(wk_s[:], wk[:])
            nc.sync.dma_start(cm_s[:], cm[:])
            nc.sync.dma_start(pb_s[:], pb[:])
            nc.sync.dma_start(bq_s[:], bqt[:])
            nc.sync.dma_start(bk_s[:], bkt[:])
            for sb in range(1, NSB):
                nc.sync.dma_start(xt_s[sb][:], xt[sb])
            for g in range(NSC):
                nc.sync.dma_start(vA[g][:, :, DK:DK + 1], ones[:, :])
            wo_s = pp.tile([P, 4, DM], bf16, tag="wo")
            nc.sync.dma_start(wo_s[:], wo[:])

            def v_strip(g):
                """V projection for sequence chunk g -> vA[g] (+bias)."""
                sb, ssc = g // 4, g % 4
                psum = psS.tile([P, 512], f32, tag="strip", name=f"pv{g}")
                for dc in range(NDC):
                    nc.tensor.matmul(
                        psum[:],
                        xt_s[sb][:, dc, ssc * P:(ssc + 1) * P],
                        wv_s[:, dc, :],
                        start=(dc == 0), stop=(dc == NDC - 1))
                nc.vector.tensor_tensor(
                    vA[g][:, :, 0:DK],
                    psum[:].rearrange("p (h d) -> p h d", h=HPC),
                    bv_s[:].rearrange("p (h d) -> p h d", h=HPC),
                    ADD)

            def qk_proj(t):
                """Q^T and K^T rows for head pair t (partitions 0..127)."""
                for (w_s, b_s, dst, nm) in ((wq_s, bq_s, qT, "q"),
                                            (wk_s, bk_s, kT, "k")):
                    for sb in range(NSB):
                        psum = psS.tile([P, 512], f32, tag="strip",
                                        name=f"p{nm}{t}_{sb}")
                        for dc in range(NDC):
                            nc.tensor.matmul(
                                psum[:],
                                w_s[:, dc, t * P:(t + 1) * P],
                                xt_s[sb][:, dc, :],
                                start=(dc == 0), stop=(dc == NDC - 1))
                        nc.vector.tensor_tensor(
                            dst[t][:, sb * 512:(sb + 1) * 512],
                            psum[:],
                            b_s[:, t:t + 1].to_broadcast([P, 512]),
                            ADD)

            def attn_block(t, qb):
                """Attention for head pair t, query block qb (512 queries).

                Heads he=2t (partitions 0:64) and ho=2t+1 (64:128) run
                concurrently in the PE array (K=64 row tiling)."""
                q0 = qb * 512
                nkc = 4 * qb + 4
                ops_e = psO.tile([DK + 1, 512], f32, tag="opse",
                                 name=f"ope{t}_{qb}")
                ops_o = psO.tile([DK + 1, 512], f32, tag="opso",
                                 name=f"opo{t}_{qb}")
                for kc in range(nkc):
                    k0 = kc * P
                    c = max(0, k0 - q0)
                    w = 512 - c
                    strip = psS.tile([P, 1024], f32, tag="strip",
                                     name=f"st{t}_{qb}_{kc}")
                    # concurrent score matmuls (row groups 0-1 / 2-3)
                    nc.tensor.matmul(
                        strip[:, c:512],
                        kT[t][0:DK, k0:k0 + P],
                        qT[t][0:DK, q0 + c:q0 + 512],
                        start=True, stop=True)
                    nc.tensor.matmul(
                        strip[:, 512 + c:1024],
                        kT[t][DK:P, k0:k0 + P],
                        qT[t][DK:P, q0 + c:q0 + 512],
                        start=True, stop=True)
                    exp_s = pe.tile([P, 1024], bf16, tag="exp",
                                    name=f"ex{t}_{qb}_{kc}")
                    # one exp for both heads via strided [P, 2, w] APs
                    nc.scalar.activation(
                        exp_s[:].rearrange("p (h q) -> p h q", h=2)[:, :, c:512],
                        strip[:].rearrange("p (h q) -> p h q", h=2)[:, :, c:512],
                        Exp, bias=pb_s[:, kc:kc + 1], scale=SCALE)
                    if k0 >= q0:  # diagonal block: causal mask multiply
                        nc.vector.tensor_tensor(
                            exp_s[:, c:c + P], exp_s[:, c:c + P], cm_s[:], MUL)
                        nc.vector.tensor_tensor(
                            exp_s[:, 512 + c:512 + c + P],
                            exp_s[:, 512 + c:512 + c + P], cm_s[:], MUL)
                    nc.tensor.matmul(
                        ops_e[:, c:512], vA[kc][:, 2 * t, :], exp_s[:, c:512],
                        start=(kc == 0), stop=(kc == nkc - 1))
                    nc.tensor.matmul(
                        ops_o[:, c:512], vA[kc][:, 2 * t + 1, :],
                        exp_s[:, 512 + c:1024],
                        start=(kc == 0), stop=(kc == nkc - 1))
                # normalisation: denominators (PSUM row 64 of each head)
                # round-trip through DRAM into a [128,8] layout so the
                # DVE reciprocal (8 cyc/elem) runs 128 lanes wide, then a
                # second round-trip broadcasts 1/den across partitions.
                den = pn.tile([1, 1024], f32, tag="den", name=f"dn{t}_{qb}")
                nc.vector.tensor_copy(den[:, 0:512], ops_e[DK:DK + 1, :])
                nc.vector.tensor_copy(den[:, 512:1024], ops_o[DK:DK + 1, :])
                sc1 = scr.tile([1, 1024], f32, tag="sc1", name=f"s1{t}_{qb}")
                nc.gpsimd.dma_start(sc1[:], den[:])
                den_t = pn.tile([P, 8], f32, tag="dent", name=f"dt{t}_{qb}")
                nc.gpsimd.dma_start(
                    den_t[:], sc1[0, :].rearrange("(p j) -> p j", p=P))
                rcp_t = pn.tile([P, 8], f32, tag="rcpt", name=f"rt{t}_{qb}")
                nc.vector.reciprocal(rcp_t[:], den_t[:])
                sc2 = scr.tile([1, 1024], f32, tag="sc2", name=f"s2{t}_{qb}")
                nc.gpsimd.dma_start(
                    sc2[0, :].rearrange("(p j) -> p j", p=P), rcp_t[:])
                bc = pn.tile([P, 512], f32, tag="bc", name=f"bc{t}_{qb}")
                nc.gpsimd.dma_start(bc[0:DK, :],
                                    sc2[0:1, 0:512].to_broadcast([DK, 512]))
                nc.gpsimd.dma_start(bc[DK:P, :],
                                    sc2[0:1, 512:1024].to_broadcast([DK, 512]))
                nc.vector.tensor_tensor(
                    outT[t][0:DK, q0:q0 + 512], ops_e[0:DK, :], bc[0:DK, :],
                    MUL)
                nc.vector.tensor_tensor(
                    outT[t][DK:P, q0:q0 + 512], ops_o[0:DK, :], bc[DK:P, :],
                    MUL)

            def c_block(qb):
                """Output projection for query block qb (4 seq chunks)."""
                for j in range(4):
                    sc = qb * 4 + j
                    y_s = py.tile([P, DM], bf16, tag="ys", name=f"ys{sc}")
                    for eh in range(2):
                        psum = psS.tile([P, 512], f32, tag="strip",
                                        name=f"py{sc}_{eh}")
                        for t in range(NPAIR):
                            nc.tensor.matmul(
                                psum[:],
                                outT[t][:, sc * P:(sc + 1) * P],
                                wo_s[:, t, eh * 512:(eh + 1) * 512],
                                start=(t == 0), stop=(t == NPAIR - 1))
                        nc.vector.tensor_copy(
                            y_s[:, eh * 512:(eh + 1) * 512], psum[:])
                    nc.sync.dma_start(y[sc * P:(sc + 1) * P, :], y_s[:])

            # ---- emission order: maximise overlap ----
            for g in range(4):          # V for kc 0..3 (needs xt0 only)
                v_strip(g)
            for t in range(NPAIR):      # interleave QK proj with qb=0 attn
                qk_proj(t)
                attn_block(t, 0)
            for qb in range(1, NQB):
                for g in range(4 * qb, 4 * qb + 4):
                    v_strip(g)
                attn_block(0, qb)
                attn_block(1, qb)
                c_block(qb - 1)         # previous block's out-proj, now safe
                attn_block(2, qb)
                attn_block(3, qb)
            c_block(0)                  # qb=0 was skipped above
            c_block(NQB - 1)

    _split_sync_waits(nc, mybir)
    return nc


def _prep_inputs(x, pad_mask, Wq, bq, Wk, bk, Wv, bv, Wo, bo):
    """Build the 8 per-core input maps."""
    import ml_dtypes
    bf16 = ml_dtypes.bfloat16

    def tile3(a, n):  # [n*128, F] -> [128, n, F] in bf16
        return np.ascontiguousarray(
            a.reshape(n, P, a.shape[1]).transpose(1, 0, 2).astype(bf16))

    cmv = (np.arange(P)[:, None] <= np.arange(P)[None, :]).astype(bf16)
    in_maps = []
    for c in range(NCORES):
        b, tp = c // 2, c % 2
        sl = slice(tp * E, (tp + 1) * E)
        xT = np.ascontiguousarray(x[b].T.astype(np.float32))
        padb = np.where(pad_mask[b, 0, 0] == 1, -1e30, 0.0).astype(np.float32)
        in_maps.append({
            "xt": np.ascontiguousarray(
                tile3(xT, NDC).reshape(P, NDC, NSB, 512)
                .transpose(2, 0, 1, 3)),
            "wq": tile3(np.ascontiguousarray(Wq.T[:, sl]), NDC),
            "wk": tile3(np.ascontiguousarray(Wk.T[:, sl]), NDC),
            "wv": tile3(np.ascontiguousarray(Wv.T[:, sl]), NDC),
            "wo": tile3(np.ascontiguousarray(Wo.T[sl, :]), 4),
            "bqt": np.ascontiguousarray(bq[sl].reshape(4, P).T),
            "bkt": np.ascontiguousarray(bk[sl].reshape(4, P).T),
            "bvb": np.ascontiguousarray(np.tile(bv[sl][None, :], (P, 1))),
            "cm": cmv,
            "ones": np.ones((P, HPC), dtype=bf16),
            "pb": np.ascontiguousarray(padb.reshape(NSC, P).T),
        })
    return in_maps


def _enable_tracing():
    """Register the NTFF profile hook (the image lacks antenv.axon_hooks)
    and neuter the bucket upload the trace path attempts."""
    import sys
    import types
    try:
        import antenv.axon_hooks  # noqa: F401
    except ImportError:
        from trn_agent_boot.trn_boot import _ntff_profile_via_ctypes
        m = types.ModuleType("antenv.axon_hooks")
        hook = _ntff_profile_via_ctypes("/opt/axon/libaxon_pjrt.so")
        m.get_axon_ntff_profile_hook = lambda: hook
        sys.modules["antenv.axon_hooks"] = m
    import concourse.bass_utils as bu
    bu.upload_artifacts = lambda tmpdir: tmpdir


def kernel_with_stats(inputs, trace=False):
    from concourse.bass_utils import run_bass_kernel_spmd

    if trace:
        try:
            _enable_tracing()
        except Exception:
            trace = False

    if "nc" not in _CACHED:
        _CACHED["nc"] = _build()
    nc = _CACHED["nc"]
    in_maps = _prep_inputs(**inputs)
    res = run_bass_kernel_spmd(nc, in_maps, core_ids=list(range(NCORES)),
                               trace=trace)
    bo = inputs["bo"].astype(np.float32)
    out = np.empty((BSZ, SEQ, DM), dtype=np.float32)
    for b in range(BSZ):
        out[b] = (res.results[2 * b]["y"].astype(np.float32)
                  + res.results[2 * b + 1]["y"].astype(np.float32) + bo)
    return out, res


def kernel(**inputs):
    out, _ = kernel_with_stats(
        inputs, trace=bool(int(os.environ.get("KERNEL_TRACE", "0"))))
    return out
